# revision 13
# baseline (speedup 1.0000x reference)
"""Causal self-attention (RoPE + RMS-norm QK, 16 heads) on 8 Trainium2 cores.

Sharding: core c = (b, g) with b = c // 4 (batch), g = c % 4 (head group of 4).
Each core computes q/k/v projections for its 4 heads from x[b], runs causal
attention, and the out-projection restricted to its head-group columns of
wproj; the host sums the 4 partial outputs per batch.

Schedule (v8):
- bf16 everywhere on the input side; fp32 only inside PSUM.
- phase P: staged start to hide the DMA ramp. tcx=0 runs the four q units
  c-pair round-robin (4 open PSUM groups) so the PE starts as soon as the
  first x/wq c-pair lands; descriptor generation is spread across the four
  engine queues (sync/scalar/vector/gpsimd each ~0.65us per dma_start).
  k then v units run unit-major with the q/k epilogues software-pipelined
  behind them; wk/wv/wp/tri DMAs are issued mid-stream, never ahead of the
  data the PE needs first. tcx 1-3 keep the v7 unit-major pipeline.
- phase D: scores in PAIRED PSUM tiles [128, 2, 512] (2 banks) so each
  off-diagonal exp is one [128,1024] ACTIVATE (amortizes the ~250ns ACT
  fixed cost); diagonal-band matmuls/exps stay column-restricted.
- causal masks: wide [128,2,512] multiplies on the otherwise-idle GpSimd
  (DVE for j=0 where mask latency is inside one head-period).
- softmax denominator: wide bf16 chain on DVE over the e pair-tiles (one
  in-place add per pair), diag pairs joined after their masks, one fold,
  then one all-ones matmul broadcasts the column sums in fp32. j=0
  accumulates directly in PSUM via column-restricted ones-matmuls.
- out-projection pieces of the previous chunk are spread into the slot
  schedule; their PSUM pool is shared with the denominator (2 banks);
  drains alternate ACT/DVE (GpSimd cannot read PSUM) and the staging
  SBUF tiles get a deep pool so out-DMA latency never throttles drains.

Per-core layout ("transposed-S"): projections produce Q^T/K^T with head-dim
on partitions, V in natural [t, d] layout. Scores are computed transposed
(S^T[tk, tq]) so softmax needs no transposes or max-subtraction (logits are
bounded by sqrt(D) after RMS-norm).
"""

import numpy as np
import ml_dtypes

import concourse.bass as bass
import concourse.mybir as mybir
import concourse.tile as tile
from concourse import bacc
from concourse.bass_utils import run_bass_kernel_spmd


def _ensure_ntff_hook():
    """If the environment requests NTFF tracing (BASS_TRACE) but the image's
    antenv lacks axon_hooks, install the same ctypes-based hook trn_boot
    would register. No-op when the real module exists."""
    import sys, types, contextlib
    try:
        from antenv.axon_hooks import get_axon_ntff_profile_hook  # noqa: F401
        return
    except ImportError:
        pass
    hook = None
    try:
        import ctypes
        lib = ctypes.CDLL("/opt/axon/libaxon_pjrt.so")
        if hasattr(lib, "axon_start_nrt_profile"):
            lib.axon_start_nrt_profile.argtypes = [
                ctypes.POINTER(ctypes.c_int64), ctypes.c_size_t]
            lib.axon_start_nrt_profile.restype = ctypes.c_int64
            lib.axon_stop_nrt_profile.argtypes = [ctypes.c_char_p]
            lib.axon_stop_nrt_profile.restype = ctypes.c_int64

            @contextlib.contextmanager
            def _hook(output_dir, device_ids):
                import jax
                jax.devices()
                if device_ids:
                    ids = (ctypes.c_int64 * len(device_ids))(*device_ids)
                    rc = lib.axon_start_nrt_profile(ids, len(device_ids))
                else:
                    rc = lib.axon_start_nrt_profile(None, 0)
                if rc != 0:
                    raise RuntimeError(f"axon_start_nrt_profile rc={rc}")
                try:
                    yield
                finally:
                    lib.axon_stop_nrt_profile(str(output_dir).encode())

            hook = _hook
    except OSError:
        pass
    import antenv
    mod = types.ModuleType("antenv.axon_hooks")
    mod.get_axon_ntff_profile_hook = lambda: hook
    mod.set_axon_ntff_profile_hook = lambda h: None
    sys.modules["antenv.axon_hooks"] = mod
    antenv.axon_hooks = mod
    # in this degraded environment there is no artifact store either
    from concourse import bass_utils
    bass_utils.upload_artifacts = lambda tmpdir: "local://" + tmpdir

P = 128          # partitions / head dim
T = 2048         # sequence length
C = 2048         # model dim
HL = 4           # heads per core
DL = HL * P      # local projection width (512)
NCO = C // P     # c-chunks (16)
XCH = 512        # x t-chunk width for projections
NXCH = T // XCH  # 4
QCH = 512        # tq-chunk width for attention
NQCH = T // QCH  # 4
NSTR = QCH // P  # diagonal-band tiles per chunk (4)
NTT = T // P     # t-tiles (16)

F32 = mybir.dt.float32
BF16 = mybir.dt.bfloat16
MUL = mybir.AluOpType.mult
SUB = mybir.AluOpType.subtract
ADD = mybir.AluOpType.add
SQRT = mybir.ActivationFunctionType.Sqrt
EXP = mybir.ActivationFunctionType.Exp


def build_program():
    nc = bacc.Bacc("TRN2", target_bir_lowering=False, debug=False, num_devices=8)

    xT = nc.dram_tensor("xT", [C, T], BF16, kind="ExternalInput")
    wqT = nc.dram_tensor("wqT", [C, DL], BF16, kind="ExternalInput")
    wkT = nc.dram_tensor("wkT", [C, DL], BF16, kind="ExternalInput")
    wvT = nc.dram_tensor("wvT", [C, DL], BF16, kind="ExternalInput")
    wpT = nc.dram_tensor("wpT", [DL, C], BF16, kind="ExternalInput")
    csA_d = nc.dram_tensor("csA", [P, T], BF16, kind="ExternalInput")   # cos|cos
    csB_d = nc.dram_tensor("csB", [P, T], BF16, kind="ExternalInput")   # sin|-sin
    tri_d = nc.dram_tensor("tri", [P, NSTR, QCH], BF16, kind="ExternalInput")
    ones_d = nc.dram_tensor("ones", [P, P], BF16, kind="ExternalInput")
    out_p = nc.dram_tensor("out_p", [T, C], BF16, kind="ExternalOutput")

    xT_r = xT.ap().rearrange("(co p) t -> p co t", p=P)

    # rotate dma_start issue across engine queues: descriptor generation is
    # ~0.65us of engine time per call, so parallelize it
    dma_engines = None  # set inside context

    with tile.TileContext(nc) as tc:
        with tc.tile_pool(name="base", bufs=1) as base:
            QT_sb = base.tile([P, HL, T], BF16, tag="QT")   # [d, h, tq]
            KT_sb = base.tile([P, HL, T], BF16, tag="KT")   # [d, h, tk]
            V_sb = base.tile([P, NTT, DL], BF16, tag="V")   # [t_sub, t_tile, d]
            ones_sb = base.tile([P, P], BF16, tag="ones")
            csA_sb = base.tile([P, T], BF16, tag="csA")
            csB_sb = base.tile([P, T], BF16, tag="csB")
            wp_sb = base.tile([P, HL, C], BF16, tag="wp")
            tri4_sb = base.tile([P, NSTR, QCH], BF16, tag="tri4")

            dma_engines = [nc.sync, nc.scalar, nc.gpsimd]

            def dma(i, dst, src):
                dma_engines[i % 3].dma_start(dst, src)

            # ---- phase P: Q/K/V projections in one pass over x --------
            with (
                tc.tile_pool(name="pw", bufs=1) as pw,
                tc.tile_pool(name="px", bufs=2) as px,
                tc.tile_pool(name="pe1", bufs=2) as pe1,
                tc.tile_pool(name="pe2", bufs=2) as pe2,
                tc.tile_pool(name="ps_acc", bufs=6, space="PSUM") as ps_acc,
                tc.tile_pool(name="ps_ssq", bufs=2, space="PSUM") as ps_ssq,
            ):
                wq_sb = pw.tile([P, NCO, DL], BF16, tag="wq")
                wk_sb = pw.tile([P, NCO, DL], BF16, tag="wk")
                wv_sb = pw.tile([P, NCO, DL], BF16, tag="wv")
                wq_r = wqT.ap().rearrange("(co p) d -> p co d", p=P)
                wk_r = wkT.ap().rearrange("(co p) d -> p co d", p=P)
                wv_r = wvT.ap().rearrange("(co p) d -> p co d", p=P)

                def project_qk(x_sb, w_sb, h):
                    psq = ps_acc.tile([P, XCH], F32, tag="acc")
                    for c in range(NCO):
                        nc.tensor.matmul(
                            psq[:],
                            w_sb[:, c, h * P : (h + 1) * P],
                            x_sb[:, c, :],
                            start=(c == 0),
                            stop=(c == NCO - 1),
                        )
                    return psq

                def project_v(x_sb, m):
                    psv = ps_acc.tile([P, DL], F32, tag="acc")
                    for c in range(NCO):
                        nc.tensor.matmul(
                            psv[:],
                            x_sb[:, c, m * P : (m + 1) * P],
                            wv_sb[:, c, :],
                            start=(c == 0),
                            stop=(c == NCO - 1),
                        )
                    return psv

                def epilogue_qk(cols, dst_sb, h, scale, psq):
                    # RoPE fully in bf16 SBUF. csA = cos|cos, csB = sin|-sin,
                    # so tmp = [-q2*sin | q1*sin] with base-aligned reads and
                    # the combine is one full-height subtract.
                    qc = pe1.tile([P, XCH], BF16, tag="qc")
                    nc.scalar.copy(qc[:], psq[:])
                    tmp = pe2.tile([P, XCH], BF16, tag="tmp")
                    lo, hi = slice(0, 64), slice(64, P)
                    nc.vector.tensor_tensor(tmp[lo, :], qc[hi, :], csB_sb[hi, cols], MUL)
                    nc.vector.tensor_tensor(tmp[hi, :], qc[lo, :], csB_sb[lo, cols], MUL)
                    qr = pe1.tile([P, XCH], BF16, tag="qr")
                    nc.vector.tensor_tensor(qr[:], qc[:], csA_sb[:, cols], MUL)
                    nc.vector.tensor_tensor(qr[:], qr[:], tmp[:], SUB)
                    # RMS: ssq broadcast over partitions via all-ones lhsT;
                    # rinv = sqrt(scale / ssq) via DVE recip + ACT sqrt
                    q2t = pe2.tile([P, XCH], BF16, tag="q2t")
                    nc.vector.tensor_tensor(q2t[:], qr[:], qr[:], MUL)
                    ssq = ps_ssq.tile([P, XCH], F32, tag="ssq")
                    nc.tensor.matmul(ssq[:], ones_sb[:], q2t[:], start=True, stop=True)
                    r1 = pe2.tile([P, XCH], F32, tag="r1")
                    nc.vector.reciprocal_approx_fast(r1[:], ssq[:])
                    rinv = pe2.tile([P, XCH], BF16, tag="rinv")
                    nc.scalar.activation(rinv[:], r1[:], SQRT, scale=scale)
                    nc.vector.tensor_tensor(dst_sb[:, h, cols], qr[:], rinv[:], MUL)

                def run_epilogue(tcx, pkind, pidx, pps):
                    cols = slice(tcx * XCH, (tcx + 1) * XCH)
                    if pkind == "q":
                        epilogue_qk(cols, QT_sb, pidx, 1.0, pps)
                    elif pkind == "k":
                        epilogue_qk(cols, KT_sb, pidx, float(P), pps)
                    else:
                        nc.scalar.copy(
                            V_sb[:, tcx * (XCH // P) + pidx, :], pps[:]
                        )

                # ---- tcx = 0: staged start -----------------------------
                cols0 = slice(0, XCH)
                x0 = px.tile([P, NCO, XCH], BF16, tag="x")
                # first two c-pairs for stage 1 on separate engine queues
                dma(0, x0[:, 0:2, :], xT_r[:, 0:2, cols0])
                dma(1, wq_sb[:, 0:2, :], wq_r[:, 0:2, :])
                dma(2, x0[:, 2:4, :], xT_r[:, 2:4, cols0])
                dma(3, wq_sb[:, 2:4, :], wq_r[:, 2:4, :])

                # stage 1: q units, c-pair round-robin over 4 open groups
                qps = [
                    ps_acc.tile([P, XCH], F32, tag="acc", name=f"qps{h}")
                    for h in range(HL)
                ]
                for cp in range(NCO // 2):
                    c0 = 2 * cp
                    if cp < 6:  # prefetch c-pair cp+2
                        dma(cp, x0[:, c0 + 4 : c0 + 6, :], xT_r[:, c0 + 4 : c0 + 6, cols0])
                        dma(cp + 1, wq_sb[:, c0 + 4 : c0 + 6, :], wq_r[:, c0 + 4 : c0 + 6, :])
                    if cp == 2:
                        dma(2, csA_sb[:], csA_d.ap())
                        dma(3, csB_sb[:], csB_d.ap())
                    if cp == 3:
                        dma(2, ones_sb[:], ones_d.ap())
                    if cp >= 4:  # wk in 4-chunk pieces
                        i4 = cp - 4
                        dma(3, wk_sb[:, 4 * i4 : 4 * i4 + 4, :], wk_r[:, 4 * i4 : 4 * i4 + 4, :])
                    for h in range(HL):
                        for c in (c0, c0 + 1):
                            nc.tensor.matmul(
                                qps[h][:],
                                wq_sb[:, c, h * P : (h + 1) * P],
                                x0[:, c, :],
                                start=(c == 0),
                                stop=(c == NCO - 1),
                            )

                # stages 2+3: k then v unit-major; q/k epilogues pipelined
                pend_epi = [("q", h, qps[h]) for h in range(HL)]
                for h in range(HL):
                    if h == 0:
                        dma(2, wv_sb[:, 0:8, :], wv_r[:, 0:8, :])
                    if h == 1:
                        dma(1, wv_sb[:, 8:16, :], wv_r[:, 8:16, :])
                    ps = project_qk(x0, wk_sb, h)
                    pend_epi.append(("k", h, ps))
                    run_epilogue(0, *pend_epi.pop(0))
                for m in range(XCH // P):
                    if m == 0:
                        dma(1, wp_sb[:], wpT.ap().rearrange("(h p) j -> p h j", p=P))
                    if m == 1:
                        dma(2, tri4_sb[:], tri_d.ap())
                    ps = project_v(x0, m)
                    pend_epi.append(("v", m, ps))
                    run_epilogue(0, *pend_epi.pop(0))
                for item in pend_epi:
                    run_epilogue(0, *item)
                pend_epi = []

                # ---- tcx = 1..3: v7 unit-major pipeline ----------------
                for tcx in range(1, NXCH):
                    cols = slice(tcx * XCH, (tcx + 1) * XCH)
                    x_sb = px.tile([P, NCO, XCH], BF16, tag="x")
                    dma(tcx, x_sb[:], xT_r[:, :, cols])

                    units = (
                        [("q", h) for h in range(HL)]
                        + [("k", h) for h in range(HL)]
                        + [("v", m) for m in range(XCH // P)]
                    )
                    # two-unit lookahead: each epilogue's ACT/DVE chain gets
                    # two projection blocks of time before its ssq matmul
                    # appears in the PE queue
                    pend_epi = []
                    for kind, idx in units:
                        if kind == "q":
                            ps = project_qk(x_sb, wq_sb, idx)
                        elif kind == "k":
                            ps = project_qk(x_sb, wk_sb, idx)
                        else:
                            ps = project_v(x_sb, idx)
                        pend_epi.append((kind, idx, ps))
                        if len(pend_epi) > 2:
                            run_epilogue(tcx, *pend_epi.pop(0))
                    for item in pend_epi:
                        run_epilogue(tcx, *item)

            # ---- phase D: attention + out-projection ------------------
            with (
                tc.tile_pool(name="de", bufs=18) as de,        # e pair-tiles
                tc.tile_pool(name="dsum", bufs=4) as dsum,     # wide esum acc
                tc.tile_pool(name="desum", bufs=3) as desum,   # folded esum
                tc.tile_pool(name="dm", bufs=2) as dm,         # otch / recip
                tc.tile_pool(name="dosb", bufs=6) as dosb,     # out staging
                tc.tile_pool(name="ps_stp", bufs=2, space="PSUM") as ps_stp,
                tc.tile_pool(name="ps_ot", bufs=2, space="PSUM") as ps_ot,
                tc.tile_pool(name="ps_po", bufs=2, space="PSUM") as ps_po,
            ):

                class HState:
                    __slots__ = ("ep", "dp", "esum2")
                    def __init__(self):
                        self.ep = []      # off-diag e pair-tiles [P,2,QCH]
                        self.dp = []      # diag pair-tiles (masked at tail)
                        self.esum2 = None # wide running sum [P,2,QCH]

                def chain_push(st, ep):
                    # wide bf16 DVE chain: first two seed, rest in-place
                    st.ep.append(ep)
                    if len(st.ep) == 1:
                        return
                    if st.esum2 is None:
                        st.esum2 = dsum.tile([P, 2, QCH], BF16, tag="esum2", name="esum2")
                        nc.vector.tensor_tensor(st.esum2[:], st.ep[0][:], ep[:], ADD)
                    else:
                        nc.vector.tensor_tensor(st.esum2[:], st.esum2[:], ep[:], ADD)

                def emit_qk_pair(j, h, ip, st):
                    """Two QK tiles into a paired PSUM tile + their exp(s)."""
                    noffp = j * 2
                    stp = ps_stp.tile([P, 2, QCH], F32, tag="st")
                    if ip < noffp:
                        i0 = 2 * ip
                        for k in (0, 1):
                            nc.tensor.matmul(
                                stp[:, k, :],
                                KT_sb[:, h, (i0 + k) * P : (i0 + k + 1) * P],
                                QT_sb[:, h, j * QCH : (j + 1) * QCH],
                                start=True,
                                stop=True,
                            )
                        ep = de.tile([P, 2, QCH], BF16, tag="e")
                        nc.scalar.activation(ep[:], stp[:], EXP)
                        chain_push(st, ep)
                    else:
                        d = ip - noffp  # 0 or 1
                        i0 = j * NSTR + 2 * d
                        dp = de.tile([P, 2, QCH], BF16, tag="e")
                        for k in (0, 1):
                            r = 2 * d + k
                            cc = slice(r * P, QCH)
                            nc.tensor.matmul(
                                stp[:, k, cc],
                                KT_sb[:, h, (i0 + k) * P : (i0 + k + 1) * P],
                                QT_sb[:, h, j * QCH + r * P : (j + 1) * QCH],
                                start=True,
                                stop=True,
                            )
                            nc.scalar.activation(dp[:, k, cc], stp[:, k, cc], EXP)
                        st.dp.append(dp)

                def qk_tail(j, h, st):
                    # diag masks + denominator joins, column-restricted so no
                    # never-written SBUF region is ever read. GpSimd (which
                    # cannot touch PSUM and is otherwise idle) takes the j>0
                    # masks and joins; j=0's go to DVE since PV needs the
                    # masked tiles within a single head-period.
                    eng = nc.vector if j == 0 else nc.gpsimd
                    for d, dp in enumerate(st.dp):
                        for k in (0, 1):
                            r = 2 * d + k
                            cc = slice(r * P, QCH)
                            eng.tensor_tensor(
                                dp[:, k, cc], dp[:, k, cc], tri4_sb[:, r, cc], MUL
                            )
                    if j > 0:
                        for d, dp in enumerate(st.dp):
                            for k in (0, 1):
                                r = 2 * d + k
                                cc = slice(r * P, QCH)
                                nc.gpsimd.tensor_tensor(
                                    st.esum2[:, k, cc],
                                    st.esum2[:, k, cc],
                                    dp[:, k, cc],
                                    ADD,
                                )

                def emit_pv(j, h, i, st, ot_ps):
                    noff = j * NSTR
                    ntk = noff + NSTR
                    r = i - noff
                    if r >= 0:
                        cc = slice(r * P, QCH)
                        e_ap = st.dp[r // 2][:, r % 2, cc]
                        ocols = cc
                    else:
                        e_ap = st.ep[i // 2][:, i % 2, :]
                        ocols = slice(0, QCH)
                    nc.tensor.matmul(
                        ot_ps[:, ocols],
                        V_sb[:, i, h * P : (h + 1) * P],
                        e_ap,
                        start=(i == 0),
                        stop=(i == ntk - 1),
                    )

                def pv_tail(j, h, st, ot_ps, ot_ch):
                    den_ps = ps_po.tile([P, QCH], F32, tag="po")
                    if j == 0:
                        # diag-only chunk: accumulate the denominator
                        # directly in PSUM, no chain to wait for
                        for r in range(NSTR):
                            cc = slice(r * P, QCH)
                            nc.tensor.matmul(
                                den_ps[:, cc],
                                ones_sb[:],
                                st.dp[r // 2][:, r % 2, cc],
                                start=(r == 0),
                                stop=(r == NSTR - 1),
                            )
                    else:
                        esum = desum.tile([P, QCH], BF16, tag="esum")
                        nc.vector.tensor_tensor(
                            esum[:], st.esum2[:, 0, :], st.esum2[:, 1, :], ADD
                        )
                        nc.tensor.matmul(den_ps[:], ones_sb[:], esum[:], start=True, stop=True)
                    recip = dm.tile([P, QCH], F32, tag="recip")
                    nc.vector.reciprocal_approx_fast(recip[:], den_ps[:])
                    nc.vector.tensor_tensor(ot_ch[:, h, :], ot_ps[:], recip[:], MUL)

                po_n = [0]

                def emit_po(jp, ot_prev, piece, drain=None):
                    u, jc = piece
                    po = ps_po.tile([P, QCH], F32, tag="po")
                    for h in range(HL):
                        nc.tensor.matmul(
                            po[:],
                            ot_prev[:, h, u * P : (u + 1) * P],
                            wp_sb[:, h, jc * QCH : (jc + 1) * QCH],
                            start=(h == 0),
                            stop=(h == HL - 1),
                        )
                    osb = dosb.tile([P, QCH], BF16, tag="osb")
                    if drain is None:
                        drain = "act" if po_n[0] % 3 == 0 else "dve"
                    po_n[0] += 1
                    if drain == "act":
                        nc.scalar.copy(osb[:], po[:])
                    else:
                        nc.vector.tensor_copy(osb[:], po[:])
                    nc.sync.dma_start(
                        out_p.ap()[
                            jp * QCH + u * P : jp * QCH + (u + 1) * P,
                            jc * QCH : (jc + 1) * QCH,
                        ],
                        osb[:],
                    )

                # Slot schedule per chunk: each slot leads with guaranteed-
                # ready work (prev head's PV, prev chunk's out-proj pieces)
                # and ends with the exp-paced QK pair, so the in-order PE
                # queue never idles on the scalar engine.
                all_pieces = [(u, jc) for u in range(QCH // P) for jc in range(NQCH)]
                prev = None  # (j, ot_ch) awaiting out-projection
                for j in range(NQCH):
                    npairs = (j + 1) * 2
                    ot_ch = dm.tile([P, HL, QCH], BF16, tag="otch")
                    pieces = list(all_pieces) if prev is not None else []
                    # front-load one piece per h=0 slot (those slots have no
                    # PV work, so the PE would otherwise run ahead of the
                    # exps), then spread the rest over the remaining slots
                    n_h0 = min(len(pieces), npairs)
                    pcadence = (HL * npairs) / max(1, len(pieces) - n_h0)
                    pacc = 0.0

                    def slot_po(h, drain=None):
                        nonlocal pacc
                        if h == 0:
                            if pieces:
                                emit_po(prev[0], prev[1], pieces.pop(0), drain=drain)
                            return
                        pacc += 1.0
                        while pieces and pacc >= pcadence:
                            pacc -= pcadence
                            emit_po(prev[0], prev[1], pieces.pop(0), drain=drain)

                    states = {}
                    ots = {}
                    for h in range(HL):
                        states[h] = HState()
                        if h >= 1:
                            ots[h - 1] = ps_ot.tile([P, QCH], F32, tag="ot", name="ot")
                        for ip in range(npairs):
                            if h >= 1:
                                emit_pv(j, h - 1, 2 * ip, states[h - 1], ots[h - 1])
                                emit_pv(j, h - 1, 2 * ip + 1, states[h - 1], ots[h - 1])
                            slot_po(h)
                            emit_qk_pair(j, h, ip, states[h])
                        if h >= 1:
                            pv_tail(j, h - 1, states[h - 1], ots[h - 1], ot_ch)
                            del states[h - 1]
                        qk_tail(j, h, states[h])
                    ot_l = ps_ot.tile([P, QCH], F32, tag="ot")
                    ots[HL - 1] = ot_l
                    for ip in range(npairs):
                        emit_pv(j, HL - 1, 2 * ip, states[HL - 1], ot_l)
                        emit_pv(j, HL - 1, 2 * ip + 1, states[HL - 1], ot_l)
                        slot_po(HL)
                    pv_tail(j, HL - 1, states[HL - 1], ot_l, ot_ch)
                    while pieces:
                        emit_po(prev[0], prev[1], pieces.pop(0))
                    prev = (j, ot_ch)
                for n, piece in enumerate(all_pieces):
                    emit_po(prev[0], prev[1], piece,
                            drain="act" if n % 2 else "dve")

    nc.compile()
    return nc


_NC = None


def _get_nc():
    global _NC
    if _NC is None:
        _NC = build_program()
    return _NC


def _host_inputs(x, cos, sin, wq, wk, wv, wproj):
    BF = ml_dtypes.bfloat16
    B = x.shape[0]
    cosT = np.ascontiguousarray(cos[0, :, 0, :].T).astype(np.float32)  # [64, T]
    sinT = np.ascontiguousarray(sin[0, :, 0, :].T).astype(np.float32)
    csA = np.concatenate([cosT, cosT], axis=0).astype(BF)
    csB = np.concatenate([sinT, -sinT], axis=0).astype(BF)
    # tri[p, r, f] = 1 iff causal (tk=128r+p <= tq=f) within a diagonal band
    rr, pp, ff = np.meshgrid(np.arange(NSTR), np.arange(P), np.arange(QCH), indexing="ij")
    tri = np.ascontiguousarray(
        (pp + 128 * rr <= ff).astype(np.float32).transpose(1, 0, 2)
    ).astype(BF)
    ones = np.ones((P, P), BF)

    xTs = [np.ascontiguousarray(x[b].T).astype(BF) for b in range(B)]
    in_maps = []
    for core in range(8):
        b, g = divmod(core, 4)
        sl = slice(g * DL, (g + 1) * DL)
        in_maps.append({
            "xT": xTs[b],
            "wqT": np.ascontiguousarray(wq[sl, :].T).astype(BF),
            "wkT": np.ascontiguousarray(wk[sl, :].T).astype(BF),
            "wvT": np.ascontiguousarray(wv[sl, :].T).astype(BF),
            "wpT": np.ascontiguousarray(wproj[:, sl].T).astype(BF),
            "csA": csA, "csB": csB, "tri": tri, "ones": ones,
        })
    return in_maps


def kernel(x, cos, sin, wq, wk, wv, wproj, _trace=False):
    _ensure_ntff_hook()
    nc = _get_nc()
    in_maps = _host_inputs(x, cos, sin, wq, wk, wv, wproj)
    res = run_bass_kernel_spmd(nc, in_maps, core_ids=list(range(8)), trace=_trace)
    parts = [res.results[c]["out_p"].astype(np.float32) for c in range(8)]
    out = np.stack([
        sum(parts[0:4]),
        sum(parts[4:8]),
    ]).astype(np.float32)
    kernel.last_exec_time_ns = res.exec_time_ns
    kernel.last_result = res
    return out


# revision 18
# speedup vs baseline: 1.0466x; 1.0466x over previous
"""Causal self-attention (RoPE + RMS-norm QK, 16 heads) on 8 Trainium2 cores.

Sharding: core c = (b, g) with b = c // 4 (batch), g = c % 4 (head group of 4).
Each core computes q/k/v projections for its 4 heads from x[b], runs causal
attention, and the out-projection restricted to its head-group columns of
wproj; the host sums the 4 partial outputs per batch.

Schedule (v8):
- bf16 everywhere on the input side; fp32 only inside PSUM.
- phase P: staged start to hide the DMA ramp. tcx=0 runs the four q units
  c-pair round-robin (4 open PSUM groups) so the PE starts as soon as the
  first x/wq c-pair lands; descriptor generation is spread across the four
  engine queues (sync/scalar/vector/gpsimd each ~0.65us per dma_start).
  k then v units run unit-major with the q/k epilogues software-pipelined
  behind them; wk/wv/wp/tri DMAs are issued mid-stream, never ahead of the
  data the PE needs first. tcx 1-3 keep the v7 unit-major pipeline.
- phase D: scores in PAIRED PSUM tiles [128, 2, 512] (2 banks) so each
  off-diagonal exp is one [128,1024] ACTIVATE (amortizes the ~250ns ACT
  fixed cost); diagonal-band matmuls/exps stay column-restricted.
- causal masks: wide [128,2,512] multiplies on the otherwise-idle GpSimd
  (DVE for j=0 where mask latency is inside one head-period).
- softmax denominator: wide bf16 chain on DVE over the e pair-tiles (one
  in-place add per pair), diag pairs joined after their masks, one fold,
  then one all-ones matmul broadcasts the column sums in fp32. j=0
  accumulates directly in PSUM via column-restricted ones-matmuls.
- out-projection pieces of the previous chunk are spread into the slot
  schedule; their PSUM pool is shared with the denominator (2 banks);
  drains alternate ACT/DVE (GpSimd cannot read PSUM) and the staging
  SBUF tiles get a deep pool so out-DMA latency never throttles drains.

Per-core layout ("transposed-S"): projections produce Q^T/K^T with head-dim
on partitions, V in natural [t, d] layout. Scores are computed transposed
(S^T[tk, tq]) so softmax needs no transposes or max-subtraction (logits are
bounded by sqrt(D) after RMS-norm).
"""

import numpy as np
import ml_dtypes

import concourse.bass as bass
import concourse.mybir as mybir
import concourse.tile as tile
from concourse import bacc
from concourse.bass_utils import run_bass_kernel_spmd


def _ensure_ntff_hook():
    """If the environment requests NTFF tracing (BASS_TRACE) but the image's
    antenv lacks axon_hooks, install the same ctypes-based hook trn_boot
    would register. No-op when the real module exists."""
    import sys, types, contextlib
    try:
        from antenv.axon_hooks import get_axon_ntff_profile_hook  # noqa: F401
        return
    except ImportError:
        pass
    hook = None
    try:
        import ctypes
        lib = ctypes.CDLL("/opt/axon/libaxon_pjrt.so")
        if hasattr(lib, "axon_start_nrt_profile"):
            lib.axon_start_nrt_profile.argtypes = [
                ctypes.POINTER(ctypes.c_int64), ctypes.c_size_t]
            lib.axon_start_nrt_profile.restype = ctypes.c_int64
            lib.axon_stop_nrt_profile.argtypes = [ctypes.c_char_p]
            lib.axon_stop_nrt_profile.restype = ctypes.c_int64

            @contextlib.contextmanager
            def _hook(output_dir, device_ids):
                import jax
                jax.devices()
                if device_ids:
                    ids = (ctypes.c_int64 * len(device_ids))(*device_ids)
                    rc = lib.axon_start_nrt_profile(ids, len(device_ids))
                else:
                    rc = lib.axon_start_nrt_profile(None, 0)
                if rc != 0:
                    raise RuntimeError(f"axon_start_nrt_profile rc={rc}")
                try:
                    yield
                finally:
                    lib.axon_stop_nrt_profile(str(output_dir).encode())

            hook = _hook
    except OSError:
        pass
    import antenv
    mod = types.ModuleType("antenv.axon_hooks")
    mod.get_axon_ntff_profile_hook = lambda: hook
    mod.set_axon_ntff_profile_hook = lambda h: None
    sys.modules["antenv.axon_hooks"] = mod
    antenv.axon_hooks = mod
    # in this degraded environment there is no artifact store either
    from concourse import bass_utils
    bass_utils.upload_artifacts = lambda tmpdir: "local://" + tmpdir

P = 128          # partitions / head dim
T = 2048         # sequence length
C = 2048         # model dim
HL = 4           # heads per core
DL = HL * P      # local projection width (512)
NCO = C // P     # c-chunks (16)
XCH = 512        # x t-chunk width for projections
NXCH = T // XCH  # 4
QCH = 512        # tq-chunk width for attention
NQCH = T // QCH  # 4
NSTR = QCH // P  # diagonal-band tiles per chunk (4)
NTT = T // P     # t-tiles (16)

F32 = mybir.dt.float32
BF16 = mybir.dt.bfloat16
MUL = mybir.AluOpType.mult
SUB = mybir.AluOpType.subtract
ADD = mybir.AluOpType.add
SQRT = mybir.ActivationFunctionType.Sqrt
EXP = mybir.ActivationFunctionType.Exp


def build_program():
    nc = bacc.Bacc("TRN2", target_bir_lowering=False, debug=False, num_devices=8)

    xT = nc.dram_tensor("xT", [C, T], BF16, kind="ExternalInput")
    wqT = nc.dram_tensor("wqT", [C, DL], BF16, kind="ExternalInput")
    wkT = nc.dram_tensor("wkT", [C, DL], BF16, kind="ExternalInput")
    wvT = nc.dram_tensor("wvT", [C, DL], BF16, kind="ExternalInput")
    wpT = nc.dram_tensor("wpT", [DL, C], BF16, kind="ExternalInput")
    csA_d = nc.dram_tensor("csA", [P, T], BF16, kind="ExternalInput")   # cos|cos
    csB_d = nc.dram_tensor("csB", [P, T], BF16, kind="ExternalInput")   # sin|-sin
    tri_d = nc.dram_tensor("tri", [P, NSTR, QCH], BF16, kind="ExternalInput")
    ones_d = nc.dram_tensor("ones", [P, P], BF16, kind="ExternalInput")
    out_p = nc.dram_tensor("out_p", [T, C], BF16, kind="ExternalOutput")

    xT_r = xT.ap().rearrange("(co p) t -> p co t", p=P)

    # rotate dma_start issue across engine queues: descriptor generation is
    # ~0.65us of engine time per call, so parallelize it
    dma_engines = None  # set inside context

    with tile.TileContext(nc) as tc:
        with tc.tile_pool(name="base", bufs=1) as base:
            QT_sb = base.tile([P, HL, T], BF16, tag="QT")   # [d, h, tq]
            KT_sb = base.tile([P, HL, T], BF16, tag="KT")   # [d, h, tk]
            V_sb = base.tile([P, NTT, DL], BF16, tag="V")   # [t_sub, t_tile, d]
            ones_sb = base.tile([P, P], BF16, tag="ones")
            csA_sb = base.tile([P, T], BF16, tag="csA")
            csB_sb = base.tile([P, T], BF16, tag="csB")
            wp_sb = base.tile([P, HL, C], BF16, tag="wp")
            tri4_sb = base.tile([P, NSTR, QCH], BF16, tag="tri4")

            # DMA issue: sync and scalar each own a hardware DGE queue (1 and
            # 10), so the x/wq stream (sync) and the weight/table stream
            # (scalar) get independent descriptor generation AND independent
            # hardware queues. GpSimd dma_start degrades to software DMA --
            # never use it.
            def dma_s(dst, src):
                nc.sync.dma_start(dst, src)

            def dma_a(dst, src):
                nc.scalar.dma_start(dst, src)

            # ---- phase P: Q/K/V projections in one pass over x --------
            with (
                tc.tile_pool(name="pw", bufs=1) as pw,
                tc.tile_pool(name="px", bufs=2) as px,
                tc.tile_pool(name="pe1", bufs=2) as pe1,
                tc.tile_pool(name="pe2", bufs=2) as pe2,
                tc.tile_pool(name="ps_acc", bufs=6, space="PSUM") as ps_acc,
                tc.tile_pool(name="ps_ssq", bufs=2, space="PSUM") as ps_ssq,
            ):
                wq_sb = pw.tile([P, NCO, DL], BF16, tag="wq")
                wk_sb = pw.tile([P, NCO, DL], BF16, tag="wk")
                wv_sb = pw.tile([P, NCO, DL], BF16, tag="wv")
                wq_r = wqT.ap().rearrange("(co p) d -> p co d", p=P)
                wk_r = wkT.ap().rearrange("(co p) d -> p co d", p=P)
                wv_r = wvT.ap().rearrange("(co p) d -> p co d", p=P)

                def project_qk(x_sb, w_sb, h):
                    psq = ps_acc.tile([P, XCH], F32, tag="acc")
                    for c in range(NCO):
                        nc.tensor.matmul(
                            psq[:],
                            w_sb[:, c, h * P : (h + 1) * P],
                            x_sb[:, c, :],
                            start=(c == 0),
                            stop=(c == NCO - 1),
                        )
                    return psq

                def project_v(x_sb, m):
                    psv = ps_acc.tile([P, DL], F32, tag="acc")
                    for c in range(NCO):
                        nc.tensor.matmul(
                            psv[:],
                            x_sb[:, c, m * P : (m + 1) * P],
                            wv_sb[:, c, :],
                            start=(c == 0),
                            stop=(c == NCO - 1),
                        )
                    return psv

                def epilogue_qk(cols, dst_sb, h, scale, psq):
                    # RoPE fully in bf16 SBUF. csA = cos|cos, csB = sin|-sin,
                    # so tmp = [-q2*sin | q1*sin] with base-aligned reads and
                    # the combine is one full-height subtract.
                    qc = pe1.tile([P, XCH], BF16, tag="qc")
                    nc.scalar.copy(qc[:], psq[:])
                    tmp = pe2.tile([P, XCH], BF16, tag="tmp")
                    lo, hi = slice(0, 64), slice(64, P)
                    nc.vector.tensor_tensor(tmp[lo, :], qc[hi, :], csB_sb[hi, cols], MUL)
                    nc.vector.tensor_tensor(tmp[hi, :], qc[lo, :], csB_sb[lo, cols], MUL)
                    qr = pe1.tile([P, XCH], BF16, tag="qr")
                    nc.vector.tensor_tensor(qr[:], qc[:], csA_sb[:, cols], MUL)
                    nc.vector.tensor_tensor(qr[:], qr[:], tmp[:], SUB)
                    # RMS: ssq broadcast over partitions via all-ones lhsT;
                    # rinv = sqrt(scale / ssq) via DVE recip + ACT sqrt
                    q2t = pe2.tile([P, XCH], BF16, tag="q2t")
                    nc.vector.tensor_tensor(q2t[:], qr[:], qr[:], MUL)
                    ssq = ps_ssq.tile([P, XCH], F32, tag="ssq")
                    nc.tensor.matmul(ssq[:], ones_sb[:], q2t[:], start=True, stop=True)
                    r1 = pe2.tile([P, XCH], F32, tag="r1")
                    nc.vector.reciprocal_approx_fast(r1[:], ssq[:])
                    rinv = pe2.tile([P, XCH], BF16, tag="rinv")
                    nc.scalar.activation(rinv[:], r1[:], SQRT, scale=scale)
                    nc.vector.tensor_tensor(dst_sb[:, h, cols], qr[:], rinv[:], MUL)

                def run_epilogue(tcx, pkind, pidx, pps):
                    cols = slice(tcx * XCH, (tcx + 1) * XCH)
                    if pkind == "q":
                        epilogue_qk(cols, QT_sb, pidx, 1.0, pps)
                    elif pkind == "k":
                        epilogue_qk(cols, KT_sb, pidx, float(P), pps)
                    else:
                        nc.scalar.copy(
                            V_sb[:, tcx * (XCH // P) + pidx, :], pps[:]
                        )

                # ---- tcx = 0: staged start -----------------------------
                cols0 = slice(0, XCH)
                x0 = px.tile([P, NCO, XCH], BF16, tag="x")
                # stage-1-critical x/wq stream on sync, in consumption order
                dma_s(x0[:, 0:2, :], xT_r[:, 0:2, cols0])
                dma_s(wq_sb[:, 0:2, :], wq_r[:, 0:2, :])
                dma_s(x0[:, 2:4, :], xT_r[:, 2:4, cols0])
                dma_s(wq_sb[:, 2:4, :], wq_r[:, 2:4, :])
                # everything stage 2+ on the scalar queue (hw queue 10),
                # in consumption order; first descriptor lands after the
                # ACT table load, which is fine for all of these
                dma_a(csA_sb[:], csA_d.ap())
                dma_a(csB_sb[:], csB_d.ap())
                dma_a(ones_sb[:], ones_d.ap())
                dma_a(wk_sb[:, 0:8, :], wk_r[:, 0:8, :])
                dma_a(wk_sb[:, 8:16, :], wk_r[:, 8:16, :])
                dma_a(wv_sb[:, 0:8, :], wv_r[:, 0:8, :])
                dma_a(wv_sb[:, 8:16, :], wv_r[:, 8:16, :])
                dma_a(wp_sb[:], wpT.ap().rearrange("(h p) j -> p h j", p=P))
                dma_a(tri4_sb[:], tri_d.ap())

                # stage 1: q units, c-pair round-robin over 4 open groups
                qps = [
                    ps_acc.tile([P, XCH], F32, tag="acc", name=f"qps{h}")
                    for h in range(HL)
                ]
                for cp in range(NCO // 2):
                    c0 = 2 * cp
                    if cp < 6:  # prefetch c-pair cp+2
                        dma_s(x0[:, c0 + 4 : c0 + 6, :], xT_r[:, c0 + 4 : c0 + 6, cols0])
                        dma_s(wq_sb[:, c0 + 4 : c0 + 6, :], wq_r[:, c0 + 4 : c0 + 6, :])
                    for h in range(HL):
                        for c in (c0, c0 + 1):
                            nc.tensor.matmul(
                                qps[h][:],
                                wq_sb[:, c, h * P : (h + 1) * P],
                                x0[:, c, :],
                                start=(c == 0),
                                stop=(c == NCO - 1),
                            )

                # stages 2+3: k then v unit-major; q/k epilogues pipelined
                pend_epi = [("q", h, qps[h]) for h in range(HL)]
                for h in range(HL):
                    ps = project_qk(x0, wk_sb, h)
                    pend_epi.append(("k", h, ps))
                    run_epilogue(0, *pend_epi.pop(0))
                for m in range(XCH // P):
                    ps = project_v(x0, m)
                    pend_epi.append(("v", m, ps))
                    run_epilogue(0, *pend_epi.pop(0))
                for item in pend_epi:
                    run_epilogue(0, *item)
                pend_epi = []

                # ---- tcx = 1..3: v7 unit-major pipeline ----------------
                for tcx in range(1, NXCH):
                    cols = slice(tcx * XCH, (tcx + 1) * XCH)
                    x_sb = px.tile([P, NCO, XCH], BF16, tag="x")
                    dma_s(x_sb[:], xT_r[:, :, cols])

                    units = (
                        [("q", h) for h in range(HL)]
                        + [("k", h) for h in range(HL)]
                        + [("v", m) for m in range(XCH // P)]
                    )
                    # two-unit lookahead: each epilogue's ACT/DVE chain gets
                    # two projection blocks of time before its ssq matmul
                    # appears in the PE queue
                    pend_epi = []
                    for kind, idx in units:
                        if kind == "q":
                            ps = project_qk(x_sb, wq_sb, idx)
                        elif kind == "k":
                            ps = project_qk(x_sb, wk_sb, idx)
                        else:
                            ps = project_v(x_sb, idx)
                        pend_epi.append((kind, idx, ps))
                        if len(pend_epi) > 2:
                            run_epilogue(tcx, *pend_epi.pop(0))
                    for item in pend_epi:
                        run_epilogue(tcx, *item)

            # ---- phase D: attention + out-projection ------------------
            with (
                tc.tile_pool(name="de", bufs=18) as de,        # e pair-tiles
                tc.tile_pool(name="dsum", bufs=4) as dsum,     # wide esum acc
                tc.tile_pool(name="desum", bufs=3) as desum,   # folded esum
                tc.tile_pool(name="dm", bufs=2) as dm,         # otch / recip
                tc.tile_pool(name="dosb", bufs=6) as dosb,     # out staging
                tc.tile_pool(name="ps_stp", bufs=2, space="PSUM") as ps_stp,
                tc.tile_pool(name="ps_ot", bufs=2, space="PSUM") as ps_ot,
                tc.tile_pool(name="ps_po", bufs=2, space="PSUM") as ps_po,
            ):

                class HState:
                    __slots__ = ("ep", "dp", "esum2")
                    def __init__(self):
                        self.ep = []      # off-diag e pair-tiles [P,2,QCH]
                        self.dp = []      # diag pair-tiles (masked at tail)
                        self.esum2 = None # wide running sum [P,2,QCH]

                def chain_push(st, ep):
                    # wide bf16 DVE chain: first two seed, rest in-place
                    st.ep.append(ep)
                    if len(st.ep) == 1:
                        return
                    if st.esum2 is None:
                        st.esum2 = dsum.tile([P, 2, QCH], BF16, tag="esum2", name="esum2")
                        nc.vector.tensor_tensor(st.esum2[:], st.ep[0][:], ep[:], ADD)
                    else:
                        nc.vector.tensor_tensor(st.esum2[:], st.esum2[:], ep[:], ADD)

                def emit_qk_pair(j, h, ip, st):
                    """Two QK tiles into a paired PSUM tile + their exp(s)."""
                    noffp = j * 2
                    stp = ps_stp.tile([P, 2, QCH], F32, tag="st")
                    if ip < noffp:
                        i0 = 2 * ip
                        for k in (0, 1):
                            nc.tensor.matmul(
                                stp[:, k, :],
                                KT_sb[:, h, (i0 + k) * P : (i0 + k + 1) * P],
                                QT_sb[:, h, j * QCH : (j + 1) * QCH],
                                start=True,
                                stop=True,
                            )
                        ep = de.tile([P, 2, QCH], BF16, tag="e")
                        nc.scalar.activation(ep[:], stp[:], EXP)
                        chain_push(st, ep)
                    else:
                        d = ip - noffp  # 0 or 1
                        i0 = j * NSTR + 2 * d
                        dp = de.tile([P, 2, QCH], BF16, tag="e")
                        for k in (0, 1):
                            r = 2 * d + k
                            cc = slice(r * P, QCH)
                            nc.tensor.matmul(
                                stp[:, k, cc],
                                KT_sb[:, h, (i0 + k) * P : (i0 + k + 1) * P],
                                QT_sb[:, h, j * QCH + r * P : (j + 1) * QCH],
                                start=True,
                                stop=True,
                            )
                            nc.scalar.activation(dp[:, k, cc], stp[:, k, cc], EXP)
                        st.dp.append(dp)

                def mask_diag(j, h, st):
                    # diag masks, column-restricted so no never-written SBUF
                    # region is ever read. GpSimd (which cannot touch PSUM
                    # and is otherwise idle) takes them for j>=2 where its
                    # ~0.7us/op latency fits inside the head-period; j<=1
                    # masks go to DVE since PV needs them sooner.
                    eng = nc.vector if j <= 1 else nc.gpsimd
                    for d, dp in enumerate(st.dp):
                        for k in (0, 1):
                            r = 2 * d + k
                            cc = slice(r * P, QCH)
                            eng.tensor_tensor(
                                dp[:, k, cc], dp[:, k, cc], tri4_sb[:, r, cc], MUL
                            )

                def qk_tail(j, h, st):
                    # denominator joins of the masked diag tiles (GpSimd has
                    # ~2 head-periods before pv_tail reads esum2)
                    if j > 0:
                        for d, dp in enumerate(st.dp):
                            for k in (0, 1):
                                r = 2 * d + k
                                cc = slice(r * P, QCH)
                                nc.gpsimd.tensor_tensor(
                                    st.esum2[:, k, cc],
                                    st.esum2[:, k, cc],
                                    dp[:, k, cc],
                                    ADD,
                                )

                def emit_pv(j, h, i, st, ot_ps):
                    noff = j * NSTR
                    ntk = noff + NSTR
                    r = i - noff
                    if r >= 0:
                        cc = slice(r * P, QCH)
                        e_ap = st.dp[r // 2][:, r % 2, cc]
                        ocols = cc
                    else:
                        e_ap = st.ep[i // 2][:, i % 2, :]
                        ocols = slice(0, QCH)
                    nc.tensor.matmul(
                        ot_ps[:, ocols],
                        V_sb[:, i, h * P : (h + 1) * P],
                        e_ap,
                        start=(i == 0),
                        stop=(i == ntk - 1),
                    )

                def pv_tail(j, h, st, ot_ps, ot_ch):
                    den_ps = ps_po.tile([P, QCH], F32, tag="po")
                    if j == 0:
                        # diag-only chunk: accumulate the denominator
                        # directly in PSUM, no chain to wait for
                        for r in range(NSTR):
                            cc = slice(r * P, QCH)
                            nc.tensor.matmul(
                                den_ps[:, cc],
                                ones_sb[:],
                                st.dp[r // 2][:, r % 2, cc],
                                start=(r == 0),
                                stop=(r == NSTR - 1),
                            )
                    else:
                        esum = desum.tile([P, QCH], BF16, tag="esum")
                        nc.vector.tensor_tensor(
                            esum[:], st.esum2[:, 0, :], st.esum2[:, 1, :], ADD
                        )
                        nc.tensor.matmul(den_ps[:], ones_sb[:], esum[:], start=True, stop=True)
                    recip = dm.tile([P, QCH], F32, tag="recip")
                    nc.vector.reciprocal_approx_fast(recip[:], den_ps[:])
                    nc.vector.tensor_tensor(ot_ch[:, h, :], ot_ps[:], recip[:], MUL)

                po_n = [0]

                def emit_po(jp, ot_prev, piece, drain=None):
                    u, jc = piece
                    po = ps_po.tile([P, QCH], F32, tag="po")
                    for h in range(HL):
                        nc.tensor.matmul(
                            po[:],
                            ot_prev[:, h, u * P : (u + 1) * P],
                            wp_sb[:, h, jc * QCH : (jc + 1) * QCH],
                            start=(h == 0),
                            stop=(h == HL - 1),
                        )
                    osb = dosb.tile([P, QCH], BF16, tag="osb")
                    if drain is None:
                        drain = "act" if po_n[0] % 3 == 0 else "dve"
                    po_n[0] += 1
                    if drain == "act":
                        nc.scalar.copy(osb[:], po[:])
                    else:
                        nc.vector.tensor_copy(osb[:], po[:])
                    nc.sync.dma_start(
                        out_p.ap()[
                            jp * QCH + u * P : jp * QCH + (u + 1) * P,
                            jc * QCH : (jc + 1) * QCH,
                        ],
                        osb[:],
                    )

                # Slot schedule per chunk: each slot leads with guaranteed-
                # ready work (prev head's PV, prev chunk's out-proj pieces)
                # and ends with the exp-paced QK pair, so the in-order PE
                # queue never idles on the scalar engine.
                all_pieces = [(u, jc) for u in range(QCH // P) for jc in range(NQCH)]
                prev = None  # (j, ot_ch) awaiting out-projection
                for j in range(NQCH):
                    npairs = (j + 1) * 2
                    ot_ch = dm.tile([P, HL, QCH], BF16, tag="otch")
                    pieces = list(all_pieces) if prev is not None else []
                    # front-load one piece per h=0 slot (those slots have no
                    # PV work, so the PE would otherwise run ahead of the
                    # exps), then spread the rest over the remaining slots
                    n_h0 = min(len(pieces), npairs)
                    pcadence = (HL * npairs) / max(1, len(pieces) - n_h0)
                    pacc = 0.0

                    def slot_po(h, drain=None):
                        nonlocal pacc
                        if h == 0:
                            if pieces:
                                emit_po(prev[0], prev[1], pieces.pop(0), drain=drain)
                            return
                        pacc += 1.0
                        while pieces and pacc >= pcadence:
                            pacc -= pcadence
                            emit_po(prev[0], prev[1], pieces.pop(0), drain=drain)

                    # emit the diag pairs first within each head stream: their
                    # exps and masks then have a full head-period of slack
                    # before PV needs them
                    ip_order = list(range(npairs - 2, npairs)) + list(range(npairs - 2))
                    states = {}
                    ots = {}
                    for h in range(HL):
                        states[h] = HState()
                        if h >= 1:
                            ots[h - 1] = ps_ot.tile([P, QCH], F32, tag="ot", name="ot")
                        for si, ip in enumerate(ip_order):
                            if h >= 1:
                                emit_pv(j, h - 1, 2 * si, states[h - 1], ots[h - 1])
                                emit_pv(j, h - 1, 2 * si + 1, states[h - 1], ots[h - 1])
                            emit_qk_pair(j, h, ip, states[h])
                            if si == 1:
                                mask_diag(j, h, states[h])
                            slot_po(h)
                        if h >= 1:
                            pv_tail(j, h - 1, states[h - 1], ots[h - 1], ot_ch)
                            del states[h - 1]
                        qk_tail(j, h, states[h])
                    ot_l = ps_ot.tile([P, QCH], F32, tag="ot")
                    ots[HL - 1] = ot_l
                    for ip in range(npairs):
                        emit_pv(j, HL - 1, 2 * ip, states[HL - 1], ot_l)
                        emit_pv(j, HL - 1, 2 * ip + 1, states[HL - 1], ot_l)
                        slot_po(HL)
                    pv_tail(j, HL - 1, states[HL - 1], ot_l, ot_ch)
                    while pieces:
                        emit_po(prev[0], prev[1], pieces.pop(0))
                    prev = (j, ot_ch)
                for n, piece in enumerate(all_pieces):
                    emit_po(prev[0], prev[1], piece,
                            drain="act" if n % 2 else "dve")

    nc.compile()
    return nc


_NC = None


def _get_nc():
    global _NC
    if _NC is None:
        _NC = build_program()
    return _NC


def _host_inputs(x, cos, sin, wq, wk, wv, wproj):
    BF = ml_dtypes.bfloat16
    B = x.shape[0]
    cosT = np.ascontiguousarray(cos[0, :, 0, :].T).astype(np.float32)  # [64, T]
    sinT = np.ascontiguousarray(sin[0, :, 0, :].T).astype(np.float32)
    csA = np.concatenate([cosT, cosT], axis=0).astype(BF)
    csB = np.concatenate([sinT, -sinT], axis=0).astype(BF)
    # tri[p, r, f] = 1 iff causal (tk=128r+p <= tq=f) within a diagonal band
    rr, pp, ff = np.meshgrid(np.arange(NSTR), np.arange(P), np.arange(QCH), indexing="ij")
    tri = np.ascontiguousarray(
        (pp + 128 * rr <= ff).astype(np.float32).transpose(1, 0, 2)
    ).astype(BF)
    ones = np.ones((P, P), BF)

    xTs = [np.ascontiguousarray(x[b].T).astype(BF) for b in range(B)]
    in_maps = []
    for core in range(8):
        b, g = divmod(core, 4)
        sl = slice(g * DL, (g + 1) * DL)
        in_maps.append({
            "xT": xTs[b],
            "wqT": np.ascontiguousarray(wq[sl, :].T).astype(BF),
            "wkT": np.ascontiguousarray(wk[sl, :].T).astype(BF),
            "wvT": np.ascontiguousarray(wv[sl, :].T).astype(BF),
            "wpT": np.ascontiguousarray(wproj[:, sl].T).astype(BF),
            "csA": csA, "csB": csB, "tri": tri, "ones": ones,
        })
    return in_maps


def kernel(x, cos, sin, wq, wk, wv, wproj, _trace=False):
    _ensure_ntff_hook()
    nc = _get_nc()
    in_maps = _host_inputs(x, cos, sin, wq, wk, wv, wproj)
    res = run_bass_kernel_spmd(nc, in_maps, core_ids=list(range(8)), trace=_trace)
    parts = [res.results[c]["out_p"].astype(np.float32) for c in range(8)]
    out = np.stack([
        sum(parts[0:4]),
        sum(parts[4:8]),
    ]).astype(np.float32)
    kernel.last_exec_time_ns = res.exec_time_ns
    kernel.last_result = res
    return out
